# revision 1
# baseline (speedup 1.0000x reference)
"""GRANMixtureBernoulli loss kernel for 8 TRN2 NeuronCores (Bass/Tile).

Strategy (per sharding hint): group each subgraph's edges on one device.
The host sorts edges by subgraph, pads every subgraph to a uniform edge
count L, and shards 512 subgraphs per core.  Per-core staged layout puts
subgraphs on SBUF partitions and edge positions on the free dimension, so
the segment sums are plain free-dim reductions:

  red_adj[s,k] = sum_pos softplus(lt) - sum_pos lt*label
  red_la[s,k]  = sum_pos la

On device (per core, per (k, seg-chunk) [128, L] tile):
  - ScalarE: u = Exp(lt); sp = Ln(u + 1) with accum_out -> sum softplus
    (no softplus ACT table exists in this toolchain; Exp+Ln share one set)
  - VectorE: tensor_tensor_reduce(mult, add) -> sum lt*label in one pass
  - VectorE: tensor_reduce(add) -> sum la
Pad values (lt=-60, label=0, la=0) make padded positions contribute 0.
A small per-chunk epilogue (log_softmax over K, logsumexp over K) runs on
DVE/ACT, the 128-partition sum of log_prob uses a ones-vector matmul, and
each core DMAs one scalar partial; the host sums 8 partials into the loss.
"""

import numpy as np
from contextlib import ExitStack

import concourse.bass as bass
import concourse.tile as tile
from concourse import mybir
from concourse.vector_clock import ScopedClock
from concourse.bass_utils import run_bass_kernel_spmd

E = 4194304
K = 20
S = 4096
N_CORES = 8
SEG_PER_CORE = S // N_CORES   # 512
SC = SEG_PER_CORE // 128      # 4 seg-chunks of 128 partitions

F32 = mybir.dt.float32
BF16 = mybir.dt.bfloat16
AF = mybir.ActivationFunctionType
ALU = mybir.AluOpType
AX = mybir.AxisListType

LT_PAD = -60.0  # exp(-60) == 0 in f32 after ln(1+u); softplus(pad) == 0


SplitDrainTileContext = tile.TileContext


def split_multi_waits(nc):
    """This walrus build accepts at most ONE sem wait per instruction.
    Hoist extra waits onto injected same-engine NoOps placed just before
    the instruction (waits execute on the issuing engine's sequencer, so
    ordering is preserved)."""
    n = 0
    for fn in nc.m.functions:
        for blk in fn.blocks:
            new = []
            changed = False
            for inst in blk.instructions:
                si = inst.sync_info
                waits = list(si.on_wait) if si and si.on_wait else []
                if len(waits) > 1:
                    changed = True
                    for w in waits[:-1]:
                        nop = mybir.InstNoOp(name=f"splitw-{n}")
                        n += 1
                        nop.engine = inst.engine
                        nop.sync_info = mybir.SyncInfo(on_wait=[w], on_update=[])
                        new.append(nop)
                    inst.sync_info = mybir.SyncInfo(
                        on_wait=[waits[-1]], on_update=list(si.on_update or []))
                new.append(inst)
            if changed:
                blk.instructions = new


def build_graph(L, reps=1, dt="f32", lac_on="act", adj="label"):
    """Build the per-core SPMD graph for padded segment length L.

    reps > 1 unrolls the whole computation reps times (identical work,
    outputs overwritten) -- used by the test harness to measure per-
    iteration HW time as a slope, cancelling fixed dispatch overheads.
    """
    DT = F32 if dt == "f32" else BF16
    nc = bass.Bass()
    lt_ext = nc.declare_dram_parameter("lt", [K, SC, 128, L], DT, isOutput=False)
    la_ext = nc.declare_dram_parameter("la", [K, SC, 128, L], DT, isOutput=False)
    lb_ext = nc.declare_dram_parameter("lb", [SC, 128, L], DT, isOutput=False)
    # "lb" holds labels (adj="label") or sign 1-2*label (adj="sign")
    cnt_ext = nc.declare_dram_parameter("cnt", [128, SC], F32, isOutput=False)
    out_ext = nc.declare_dram_parameter("out", [1, 1], F32, isOutput=True)

    with SplitDrainTileContext(nc) as tc, ExitStack() as ctx:
        const_p = ctx.enter_context(tc.tile_pool(name="const", bufs=1))
        lt_p = ctx.enter_context(tc.tile_pool(name="lt", bufs=3))
        la_p = ctx.enter_context(tc.tile_pool(name="la", bufs=3))
        u_p = ctx.enter_context(tc.tile_pool(name="u", bufs=2))
        sp_p = ctx.enter_context(tc.tile_pool(name="sp", bufs=2))
        pr_p = ctx.enter_context(tc.tile_pool(name="pr", bufs=2))
        acc_p = ctx.enter_context(tc.tile_pool(name="acc", bufs=2))
        epi_p = ctx.enter_context(tc.tile_pool(name="epi", bufs=2))
        ps_p = ctx.enter_context(tc.tile_pool(name="ps", bufs=1, space="PSUM"))

        # Resident tiles: labels per seg-chunk, counts, ones vector.
        lb_t = []
        for sc in range(SC):
            t = const_p.tile([128, L], DT, tag=f"lb{sc}", name=f"lb{sc}")
            nc.sync.dma_start(t[:], lb_ext[sc])
            lb_t.append(t)
        cnt_t = const_p.tile([128, SC], F32, tag="cnt")
        nc.sync.dma_start(cnt_t[:], cnt_ext[:])
        ones_t = const_p.tile([128, 1], F32, tag="ones")
        nc.vector.memset(ones_t[:], 1.0)

        for _ in range(reps):
            a1 = [acc_p.tile([128, K], F32, tag=f"a1_{sc}", name=f"a1_{sc}")
                  for sc in range(SC)]
            a2 = [acc_p.tile([128, K], F32, tag=f"a2_{sc}", name=f"a2_{sc}")
                  for sc in range(SC)]
            lac = [acc_p.tile([128, K], F32, tag=f"lac_{sc}", name=f"lac_{sc}")
                   for sc in range(SC)]

            for k in range(K):
                for sc in range(SC):
                    lt_t = lt_p.tile([128, L], DT)
                    nc.sync.dma_start(lt_t[:], lt_ext[k, sc])
                    la_t = la_p.tile([128, L], DT)
                    nc.sync.dma_start(la_t[:], la_ext[k, sc])

                    if adj in ("sign", "tree"):
                        t_t = pr_p.tile([128, L], DT, tag="tsgn", name="tsgn")
                        nc.vector.tensor_mul(t_t[:], lt_t[:], lb_t[sc][:])
                        src_t = t_t
                    else:
                        src_t = lt_t
                    u_t = u_p.tile([128, L], DT)
                    nc.scalar.activation(u_t[:], src_t[:], AF.Exp)
                    if adj == "tree":
                        # sum ln(1+v) = sum over blocks of ln prod(1+v):
                        # 3 pair-multiply levels -> Ln pass on L/8 elements
                        w_t = sp_p.tile([128, L], DT, tag="w", name="w")
                        nc.vector.tensor_scalar_add(w_t[:], u_t[:], 1.0)
                        h1 = sp_p.tile([128, L // 2], DT, tag="h1", name="h1")
                        nc.vector.tensor_mul(h1[:], w_t[:, :L // 2],
                                             w_t[:, L // 2:])
                        h2 = sp_p.tile([128, L // 4], DT, tag="h2", name="h2")
                        nc.vector.tensor_mul(h2[:], h1[:, :L // 4],
                                             h1[:, L // 4:])
                        h3 = sp_p.tile([128, L // 8], DT, tag="h3", name="h3")
                        nc.vector.tensor_mul(h3[:], h2[:, :L // 8],
                                             h2[:, L // 8:])
                        sp_t = sp_p.tile([128, L // 8], DT, tag="sp8",
                                         name="sp8")
                        nc.scalar.activation(sp_t[:], h3[:], AF.Ln,
                                             accum_out=a1[sc][:, k:k + 1])
                    else:
                        sp_t = sp_p.tile([128, L], DT)
                        nc.scalar.activation(sp_t[:], u_t[:], AF.Ln, bias=1.0,
                                             accum_out=a1[sc][:, k:k + 1])

                    if adj == "label":
                        pr_t = pr_p.tile([128, L], DT)
                        nc.vector.tensor_mul(pr_t[:], lt_t[:], lb_t[sc][:])
                        nc.vector.tensor_reduce(
                            out=a2[sc][:, k:k + 1], in_=pr_t[:],
                            axis=AX.X, op=ALU.add)

                    if lac_on == "act":
                        lac_t = sp_p.tile([128, L], DT, tag="lacopy",
                                          name="lacopy")
                        nc.scalar.activation(lac_t[:], la_t[:], AF.Copy,
                                             accum_out=lac[sc][:, k:k + 1])
                    else:
                        nc.vector.tensor_reduce(
                            out=lac[sc][:, k:k + 1], in_=la_t[:],
                            axis=AX.X, op=ALU.add)

            # epilogue: per seg-chunk log_softmax over K + logsumexp over K
            lp_total = epi_p.tile([128, SC], F32, tag="lp_total")
            for sc in range(SC):
                if adj in ("sign", "tree"):
                    radj = a1[sc]
                else:
                    radj = epi_p.tile([128, K], F32, tag="radj")
                    nc.vector.tensor_sub(radj[:], a1[sc][:], a2[sc][:])
                cinv = epi_p.tile([128, 1], F32, tag="cinv")
                nc.vector.reciprocal(cinv[:], cnt_t[:, sc:sc + 1])
                rla = epi_p.tile([128, K], F32, tag="rla")
                nc.vector.tensor_scalar_mul(rla[:], lac[sc][:], cinv[:])

                m1n = epi_p.tile([128, 1], F32, tag="m1n")
                nc.vector.tensor_reduce(out=m1n[:], in_=rla[:], axis=AX.X,
                                        op=ALU.max, negate=True)
                e1 = epi_p.tile([128, K], F32, tag="e1")
                s1 = epi_p.tile([128, 1], F32, tag="s1")
                nc.scalar.activation(e1[:], rla[:], AF.Exp, bias=m1n[:],
                                     accum_out=s1[:])
                l1 = epi_p.tile([128, 1], F32, tag="l1")
                nc.scalar.activation(l1[:], s1[:], AF.Ln)
                b = epi_p.tile([128, 1], F32, tag="b")
                nc.vector.tensor_sub(b[:], m1n[:], l1[:])

                t1 = epi_p.tile([128, K], F32, tag="t1")
                nc.vector.tensor_sub(t1[:], rla[:], radj[:])
                z = epi_p.tile([128, K], F32, tag="z")
                nc.vector.tensor_scalar_add(z[:], t1[:], b[:])

                m2n = epi_p.tile([128, 1], F32, tag="m2n")
                nc.vector.tensor_reduce(out=m2n[:], in_=z[:], axis=AX.X,
                                        op=ALU.max, negate=True)
                e2 = epi_p.tile([128, K], F32, tag="e2")
                s2 = epi_p.tile([128, 1], F32, tag="s2")
                nc.scalar.activation(e2[:], z[:], AF.Exp, bias=m2n[:],
                                     accum_out=s2[:])
                l2 = epi_p.tile([128, 1], F32, tag="l2")
                nc.scalar.activation(l2[:], s2[:], AF.Ln)
                nc.vector.tensor_sub(lp_total[:, sc:sc + 1], l2[:], m2n[:])

            row = epi_p.tile([128, 1], F32, tag="row")
            nc.vector.tensor_reduce(out=row[:], in_=lp_total[:], axis=AX.X,
                                    op=ALU.add)
            ps_t = ps_p.tile([1, 1], F32, tag="ps")
            nc.tensor.matmul(ps_t[:], ones_t[:], row[:],
                             start=True, stop=True)
            res_t = epi_p.tile([1, 1], F32, tag="res")
            nc.vector.tensor_copy(res_t[:], ps_t[:])
            nc.sync.dma_start(out_ext[:], res_t[:])

    split_multi_waits(nc)
    return nc


def stage_inputs(label, log_theta, log_alpha, subgraph_idx, dt="f32",
                 adj="label"):
    """Sort/pad/shard the inputs into the per-core staged layout.

    Returns (in_maps, L): in_maps[c] feeds core c.
    """
    label = np.asarray(label, np.float32)
    log_theta = np.ascontiguousarray(np.asarray(log_theta, np.float32))
    log_alpha = np.ascontiguousarray(np.asarray(log_alpha, np.float32))
    idx = np.asarray(subgraph_idx).astype(np.int64)

    counts = np.bincount(idx, minlength=S).astype(np.int64)
    L = int(counts.max())
    L = (L + 15) // 16 * 16  # align free dim

    order = np.argsort(idx, kind="stable").astype(np.int64)
    starts = np.zeros(S, np.int64)
    np.cumsum(counts[:-1], out=starts[1:])
    pos_in_seg = np.arange(E, dtype=np.int64) - starts[idx[order]]
    eidx = np.full((S, L), E, dtype=np.int64)
    eidx[idx[order], pos_in_seg] = order

    ltx = np.vstack([log_theta, np.full((1, K), LT_PAD, np.float32)])
    lax = np.vstack([log_alpha, np.zeros((1, K), np.float32)])
    if adj in ("sign", "tree"):
        lbx = np.concatenate([1.0 - 2.0 * label, np.ones(1, np.float32)])
    else:
        lbx = np.concatenate([label, np.zeros(1, np.float32)])

    # [S, L, K] -> [cores, K, SC, 128, L]
    lt_g = ltx[eidx]          # [4096, L, 20]
    lt_g = lt_g.reshape(N_CORES, SC, 128, L, K).transpose(0, 4, 1, 2, 3)
    lt_g = np.ascontiguousarray(lt_g)
    la_g = lax[eidx]
    la_g = la_g.reshape(N_CORES, SC, 128, L, K).transpose(0, 4, 1, 2, 3)
    la_g = np.ascontiguousarray(la_g)
    lb_g = np.ascontiguousarray(lbx[eidx].reshape(N_CORES, SC, 128, L))
    cnt_g = np.ascontiguousarray(
        counts.astype(np.float32).reshape(N_CORES, SC, 128).transpose(0, 2, 1))

    if dt == "bf16":
        import ml_dtypes
        bf = ml_dtypes.bfloat16
        lt_g = lt_g.astype(bf)
        la_g = la_g.astype(bf)
        lb_g = lb_g.astype(bf)

    in_maps = [
        {"lt": lt_g[c], "la": la_g[c], "lb": lb_g[c], "cnt": cnt_g[c]}
        for c in range(N_CORES)
    ]
    return in_maps, L


def finish(partials):
    """Combine the 8 per-core partial sums into the scalar loss."""
    total = np.sum([np.float64(p) for p in partials])
    return np.float32(-total / E)


def kernel(label, log_theta, log_alpha, subgraph_idx):
    in_maps, Ls = stage_inputs_v2(label, log_theta, log_alpha, subgraph_idx,
                                  la_fp8=True)
    nc = build_graph_v3(Ls, la_fp8=True)
    res = run_bass_kernel_spmd(nc, in_maps, core_ids=list(range(N_CORES)))
    return finish([res.results[c]["out"][0, 0] for c in range(N_CORES)])


# ---------------------------------------------------------------------------
# v2: variable per-slot padding (segments sorted by size into 128-blocks) and
# optional fp8 staging for log_alpha.  Cuts padded bytes/compute ~8% and la
# bytes 2x.
# ---------------------------------------------------------------------------

FP8 = mybir.dt.float8e4


def _round16(x):
    return (int(x) + 15) // 16 * 16


def stage_inputs_v2(label, log_theta, log_alpha, subgraph_idx,
                    la_fp8=False):
    """Sort segments by size desc into 32 blocks of 128; block j -> core j%8,
    slot j//8.  Slot s padded length L_s = max count in blocks 8s..8s+7.
    Staged layout per core: lt/la [K, 128, Ltot], sgn [128, Ltot] where the
    free dim concatenates the 4 slots; cnt [128, 4]."""
    import ml_dtypes
    label = np.asarray(label, np.float32)
    log_theta = np.ascontiguousarray(np.asarray(log_theta, np.float32))
    log_alpha = np.ascontiguousarray(np.asarray(log_alpha, np.float32))
    idx = np.asarray(subgraph_idx).astype(np.int64)

    counts = np.bincount(idx, minlength=S).astype(np.int64)
    seg_order = np.argsort(-counts, kind="stable")

    Ls = [_round16(counts[seg_order[128 * 8 * s]]) for s in range(SC)]
    offs = np.concatenate([[0], np.cumsum(Ls)]).astype(np.int64)
    Ltot = int(offs[-1])
    Lmax = Ls[0]

    order = np.argsort(idx, kind="stable").astype(np.int64)
    starts = np.zeros(S, np.int64)
    np.cumsum(counts[:-1], out=starts[1:])
    pos_in_seg = np.arange(E, dtype=np.int64) - starts[idx[order]]
    eidx = np.full((S, Lmax), E, dtype=np.int64)
    eidx[idx[order], pos_in_seg] = order

    ltx = np.vstack([log_theta, np.full((1, K), LT_PAD, np.float32)])
    lax = np.vstack([log_alpha, np.zeros((1, K), np.float32)])
    sgx = np.concatenate([1.0 - 2.0 * label, np.ones(1, np.float32)])

    bf = ml_dtypes.bfloat16
    la_dt = ml_dtypes.float8_e4m3 if la_fp8 else bf
    lt_g = np.empty((N_CORES, K, 128, Ltot), bf)
    la_g = np.empty((N_CORES, K, 128, Ltot), la_dt)
    sg_g = np.empty((N_CORES, 128, Ltot), bf)
    cnt_g = np.empty((N_CORES, 128, SC), np.float32)

    for s in range(SC):
        lo, hi = int(offs[s]), int(offs[s + 1])
        for c in range(N_CORES):
            segs = seg_order[128 * (8 * s + c): 128 * (8 * s + c) + 128]
            ei = eidx[segs, :Ls[s]]
            lt_g[c, :, :, lo:hi] = ltx[ei].transpose(2, 0, 1).astype(bf)
            la_g[c, :, :, lo:hi] = lax[ei].transpose(2, 0, 1).astype(la_dt)
            sg_g[c, :, lo:hi] = sgx[ei].astype(bf)
            cnt_g[c, :, s] = counts[segs].astype(np.float32)

    in_maps = [
        {"lt": lt_g[c], "la": la_g[c], "lb": sg_g[c], "cnt": cnt_g[c]}
        for c in range(N_CORES)
    ]
    return in_maps, Ls


def build_graph_v2(Ls, reps=1, la_fp8=False, nbufs=6):
    """Product-tree softplus, sign-folded labels, bf16 (la optionally fp8),
    variable per-slot padded lengths."""
    offs = [0]
    for l in Ls:
        offs.append(offs[-1] + l)
    Ltot = offs[-1]
    LA_DT = FP8 if la_fp8 else BF16

    nc = bass.Bass()
    lt_ext = nc.declare_dram_parameter("lt", [K, 128, Ltot], BF16,
                                       isOutput=False)
    la_ext = nc.declare_dram_parameter("la", [K, 128, Ltot], LA_DT,
                                       isOutput=False)
    lb_ext = nc.declare_dram_parameter("lb", [128, Ltot], BF16,
                                       isOutput=False)
    cnt_ext = nc.declare_dram_parameter("cnt", [128, SC], F32, isOutput=False)
    out_ext = nc.declare_dram_parameter("out", [1, 1], F32, isOutput=True)

    with tile.TileContext(nc) as tc, ExitStack() as ctx:
        const_p = ctx.enter_context(tc.tile_pool(name="const", bufs=1))
        lt_p = ctx.enter_context(tc.tile_pool(name="lt", bufs=nbufs + 1))
        la_p = ctx.enter_context(tc.tile_pool(name="la", bufs=nbufs + 1))
        u_p = ctx.enter_context(tc.tile_pool(name="u", bufs=nbufs))
        sp_p = ctx.enter_context(tc.tile_pool(name="sp", bufs=nbufs))
        pr_p = ctx.enter_context(tc.tile_pool(name="pr", bufs=nbufs))
        acc_p = ctx.enter_context(tc.tile_pool(name="acc", bufs=2))
        epi_p = ctx.enter_context(tc.tile_pool(name="epi", bufs=2))
        ps_p = ctx.enter_context(tc.tile_pool(name="ps", bufs=1, space="PSUM"))

        sgn_t = const_p.tile([128, Ltot], BF16, tag="sgn")
        nc.sync.dma_start(sgn_t[:], lb_ext[:])
        cnt_t = const_p.tile([128, SC], F32, tag="cnt")
        nc.sync.dma_start(cnt_t[:], cnt_ext[:])
        ones_t = const_p.tile([128, 1], F32, tag="ones")
        nc.vector.memset(ones_t[:], 1.0)

        for _ in range(reps):
            a1 = [acc_p.tile([128, K], F32, tag=f"a1_{sc}", name=f"a1_{sc}")
                  for sc in range(SC)]
            lac = [acc_p.tile([128, K], F32, tag=f"lac_{sc}", name=f"lac_{sc}")
                   for sc in range(SC)]

            for k in range(K):
                for sc in range(SC):
                    L = Ls[sc]
                    lo = offs[sc]
                    lt_t = lt_p.tile([128, L], BF16)
                    nc.sync.dma_start(lt_t[:], lt_ext[k, :, lo:lo + L])
                    la_t = la_p.tile([128, L], LA_DT)
                    nc.sync.dma_start(la_t[:], la_ext[k, :, lo:lo + L])

                    t_t = pr_p.tile([128, L], BF16, tag="tsgn", name="tsgn")
                    nc.vector.tensor_mul(t_t[:], lt_t[:],
                                         sgn_t[:, lo:lo + L])
                    u_t = u_p.tile([128, L], BF16)
                    nc.scalar.activation(u_t[:], t_t[:], AF.Exp)
                    w_t = sp_p.tile([128, L], BF16, tag="w", name="w")
                    nc.vector.tensor_scalar_add(w_t[:], u_t[:], 1.0)
                    h1 = sp_p.tile([128, L // 2], BF16, tag="h1", name="h1")
                    nc.vector.tensor_mul(h1[:], w_t[:, :L // 2],
                                         w_t[:, L // 2:])
                    h2 = sp_p.tile([128, L // 4], BF16, tag="h2", name="h2")
                    nc.vector.tensor_mul(h2[:], h1[:, :L // 4],
                                         h1[:, L // 4:])
                    h3 = sp_p.tile([128, L // 8], BF16, tag="h3", name="h3")
                    nc.vector.tensor_mul(h3[:], h2[:, :L // 8],
                                         h2[:, L // 8:])
                    sp_t = sp_p.tile([128, L // 8], BF16, tag="sp8",
                                     name="sp8")
                    nc.scalar.activation(sp_t[:], h3[:], AF.Ln,
                                         accum_out=a1[sc][:, k:k + 1])

                    nc.vector.tensor_reduce(
                        out=lac[sc][:, k:k + 1], in_=la_t[:],
                        axis=AX.X, op=ALU.add)

            lp_total = epi_p.tile([128, SC], F32, tag="lp_total")
            for sc in range(SC):
                radj = a1[sc]
                cinv = epi_p.tile([128, 1], F32, tag="cinv")
                nc.vector.reciprocal(cinv[:], cnt_t[:, sc:sc + 1])
                rla = epi_p.tile([128, K], F32, tag="rla")
                nc.vector.tensor_scalar_mul(rla[:], lac[sc][:], cinv[:])

                m1n = epi_p.tile([128, 1], F32, tag="m1n")
                nc.vector.tensor_reduce(out=m1n[:], in_=rla[:], axis=AX.X,
                                        op=ALU.max, negate=True)
                e1 = epi_p.tile([128, K], F32, tag="e1")
                s1 = epi_p.tile([128, 1], F32, tag="s1")
                nc.scalar.activation(e1[:], rla[:], AF.Exp, bias=m1n[:],
                                     accum_out=s1[:])
                l1 = epi_p.tile([128, 1], F32, tag="l1")
                nc.scalar.activation(l1[:], s1[:], AF.Ln)
                b = epi_p.tile([128, 1], F32, tag="b")
                nc.vector.tensor_sub(b[:], m1n[:], l1[:])

                t1 = epi_p.tile([128, K], F32, tag="t1")
                nc.vector.tensor_sub(t1[:], rla[:], radj[:])
                z = epi_p.tile([128, K], F32, tag="z")
                nc.vector.tensor_scalar_add(z[:], t1[:], b[:])

                m2n = epi_p.tile([128, 1], F32, tag="m2n")
                nc.vector.tensor_reduce(out=m2n[:], in_=z[:], axis=AX.X,
                                        op=ALU.max, negate=True)
                e2 = epi_p.tile([128, K], F32, tag="e2")
                s2 = epi_p.tile([128, 1], F32, tag="s2")
                nc.scalar.activation(e2[:], z[:], AF.Exp, bias=m2n[:],
                                     accum_out=s2[:])
                l2 = epi_p.tile([128, 1], F32, tag="l2")
                nc.scalar.activation(l2[:], s2[:], AF.Ln)
                nc.vector.tensor_sub(lp_total[:, sc:sc + 1], l2[:], m2n[:])

            row = epi_p.tile([128, 1], F32, tag="row")
            nc.vector.tensor_reduce(out=row[:], in_=lp_total[:], axis=AX.X,
                                    op=ALU.add)
            ps_t = ps_p.tile([1, 1], F32, tag="ps")
            nc.tensor.matmul(ps_t[:], ones_t[:], row[:],
                             start=True, stop=True)
            res_t = epi_p.tile([1, 1], F32, tag="res")
            nc.vector.tensor_copy(res_t[:], ps_t[:])
            nc.sync.dma_start(out_ext[:], res_t[:])

    split_multi_waits(nc)
    return nc


def build_graph_v3(Ls, reps=1, nbufs=6, la_fp8=False):
    """v2 + k-pair batching: each tile covers two k-slices [128, 2*L_s],
    halving instruction counts (DMA, exp, mult, tree) at identical bytes.
    The pair-multiply tree uses strided 3D views so k's never mix; only the
    small Ln+accum and la reduces split per k."""
    offs = [0]
    for l in Ls:
        offs.append(offs[-1] + l)
    Ltot = offs[-1]

    LA_DT = FP8 if la_fp8 else BF16
    nc = bass.Bass()
    lt_ext = nc.declare_dram_parameter("lt", [K, 128, Ltot], BF16,
                                       isOutput=False)
    la_ext = nc.declare_dram_parameter("la", [K, 128, Ltot], LA_DT,
                                       isOutput=False)
    lb_ext = nc.declare_dram_parameter("lb", [128, Ltot], BF16,
                                       isOutput=False)
    cnt_ext = nc.declare_dram_parameter("cnt", [128, SC], F32, isOutput=False)
    out_ext = nc.declare_dram_parameter("out", [1, 1], F32, isOutput=True)

    def pairs(t, half):
        # flat [128, 2*half] -> [128, 2, half] (k-major halves)
        return t.rearrange("p (k l) -> p k l", k=2)

    with tile.TileContext(nc) as tc, ExitStack() as ctx:
        const_p = ctx.enter_context(tc.tile_pool(name="const", bufs=1))
        lt_p = ctx.enter_context(tc.tile_pool(name="lt", bufs=nbufs + 1))
        la_p = ctx.enter_context(tc.tile_pool(name="la", bufs=nbufs + 1))
        u_p = ctx.enter_context(tc.tile_pool(name="u", bufs=nbufs))
        sp_p = ctx.enter_context(tc.tile_pool(name="sp", bufs=nbufs))
        pr_p = ctx.enter_context(tc.tile_pool(name="pr", bufs=nbufs))
        acc_p = ctx.enter_context(tc.tile_pool(name="acc", bufs=2))
        epi_p = ctx.enter_context(tc.tile_pool(name="epi", bufs=2))
        ps_p = ctx.enter_context(tc.tile_pool(name="ps", bufs=1, space="PSUM"))

        # sign tile duplicated so a [128, 2, L] view exists for pair ops
        sgn2_t = const_p.tile([128, 2 * Ltot], BF16, tag="sgn2")
        nc.sync.dma_start(sgn2_t[:, :Ltot], lb_ext[:])
        nc.sync.dma_start(sgn2_t[:, Ltot:], lb_ext[:])
        cnt_t = const_p.tile([128, SC], F32, tag="cnt")
        nc.sync.dma_start(cnt_t[:], cnt_ext[:])
        ones_t = const_p.tile([128, 1], F32, tag="ones")
        nc.vector.memset(ones_t[:], 1.0)

        for _ in range(reps):
            a1 = [acc_p.tile([128, K], F32, tag=f"a1_{sc}", name=f"a1_{sc}")
                  for sc in range(SC)]
            lac = [acc_p.tile([128, K], F32, tag=f"lac_{sc}", name=f"lac_{sc}")
                   for sc in range(SC)]

            for pk in range(K // 2):
                k0 = 2 * pk
                for sc in range(SC):
                    L = Ls[sc]
                    lo = offs[sc]
                    src_lt = lt_ext[k0:k0 + 2, :, lo:lo + L].rearrange(
                        "k p l -> p k l")
                    lt2 = lt_p.tile([128, 2 * L], BF16)
                    nc.sync.dma_start(pairs(lt2, L)[:], src_lt)
                    src_la = la_ext[k0:k0 + 2, :, lo:lo + L].rearrange(
                        "k p l -> p k l")
                    la2 = la_p.tile([128, 2 * L], LA_DT)
                    nc.sync.dma_start(pairs(la2, L)[:], src_la)

                    sgn_v = pairs(sgn2_t, Ltot)[:, :, lo:lo + L]
                    t2 = pr_p.tile([128, 2 * L], BF16, tag="tsgn",
                                   name="tsgn")
                    nc.vector.tensor_mul(pairs(t2, L)[:],
                                         pairs(lt2, L)[:], sgn_v)
                    u2 = u_p.tile([128, 2 * L], BF16)
                    nc.scalar.activation(u2[:], t2[:], AF.Exp)
                    w2 = sp_p.tile([128, 2 * L], BF16, tag="w", name="w")
                    nc.vector.tensor_scalar_add(w2[:], u2[:], 1.0)

                    h1 = sp_p.tile([128, L], BF16, tag="h1", name="h1")
                    nc.vector.tensor_mul(
                        pairs(h1, L // 2)[:],
                        pairs(w2, L)[:, :, :L // 2],
                        pairs(w2, L)[:, :, L // 2:])
                    h2 = sp_p.tile([128, L // 2], BF16, tag="h2", name="h2")
                    nc.vector.tensor_mul(
                        pairs(h2, L // 4)[:],
                        pairs(h1, L // 2)[:, :, :L // 4],
                        pairs(h1, L // 2)[:, :, L // 4:])
                    h3 = sp_p.tile([128, L // 4], BF16, tag="h3", name="h3")
                    nc.vector.tensor_mul(
                        pairs(h3, L // 8)[:],
                        pairs(h2, L // 4)[:, :, :L // 8],
                        pairs(h2, L // 4)[:, :, L // 8:])

                    for j in range(2):
                        sp_t = sp_p.tile([128, L // 8], BF16, tag="sp8",
                                         name="sp8")
                        nc.scalar.activation(
                            sp_t[:], h3[:, j * (L // 8):(j + 1) * (L // 8)],
                            AF.Ln, accum_out=a1[sc][:, k0 + j:k0 + j + 1])
                        nc.vector.tensor_reduce(
                            out=lac[sc][:, k0 + j:k0 + j + 1],
                            in_=la2[:, j * L:(j + 1) * L],
                            axis=AX.X, op=ALU.add)

            lp_total = epi_p.tile([128, SC], F32, tag="lp_total")
            for sc in range(SC):
                radj = a1[sc]
                cinv = epi_p.tile([128, 1], F32, tag="cinv")
                nc.vector.reciprocal(cinv[:], cnt_t[:, sc:sc + 1])
                rla = epi_p.tile([128, K], F32, tag="rla")
                nc.vector.tensor_scalar_mul(rla[:], lac[sc][:], cinv[:])
                m1n = epi_p.tile([128, 1], F32, tag="m1n")
                nc.vector.tensor_reduce(out=m1n[:], in_=rla[:], axis=AX.X,
                                        op=ALU.max, negate=True)
                e1 = epi_p.tile([128, K], F32, tag="e1")
                s1 = epi_p.tile([128, 1], F32, tag="s1")
                nc.scalar.activation(e1[:], rla[:], AF.Exp, bias=m1n[:],
                                     accum_out=s1[:])
                l1 = epi_p.tile([128, 1], F32, tag="l1")
                nc.scalar.activation(l1[:], s1[:], AF.Ln)
                b = epi_p.tile([128, 1], F32, tag="b")
                nc.vector.tensor_sub(b[:], m1n[:], l1[:])
                t1 = epi_p.tile([128, K], F32, tag="t1")
                nc.vector.tensor_sub(t1[:], rla[:], radj[:])
                z = epi_p.tile([128, K], F32, tag="z")
                nc.vector.tensor_scalar_add(z[:], t1[:], b[:])
                m2n = epi_p.tile([128, 1], F32, tag="m2n")
                nc.vector.tensor_reduce(out=m2n[:], in_=z[:], axis=AX.X,
                                        op=ALU.max, negate=True)
                e2 = epi_p.tile([128, K], F32, tag="e2")
                s2 = epi_p.tile([128, 1], F32, tag="s2")
                nc.scalar.activation(e2[:], z[:], AF.Exp, bias=m2n[:],
                                     accum_out=s2[:])
                l2 = epi_p.tile([128, 1], F32, tag="l2")
                nc.scalar.activation(l2[:], s2[:], AF.Ln)
                nc.vector.tensor_sub(lp_total[:, sc:sc + 1], l2[:], m2n[:])

            row = epi_p.tile([128, 1], F32, tag="row")
            nc.vector.tensor_reduce(out=row[:], in_=lp_total[:], axis=AX.X,
                                    op=ALU.add)
            ps_t = ps_p.tile([1, 1], F32, tag="ps")
            nc.tensor.matmul(ps_t[:], ones_t[:], row[:],
                             start=True, stop=True)
            res_t = epi_p.tile([1, 1], F32, tag="res")
            nc.vector.tensor_copy(res_t[:], ps_t[:])
            nc.sync.dma_start(out_ext[:], res_t[:])

    split_multi_waits(nc)
    return nc

